# revision 28
# baseline (speedup 1.0000x reference)
"""BERT encoder (12 layers, B=8, S=512, H=768, NH=12, FF=3072) on 8 TRN2
NeuronCores. Data-parallel over batch: each core runs the full 12-layer
encoder on one batch element; no collectives.

v2 on-chip strategy (per core, per layer):
- Activations feature-major in SBUF: x [H=768 (6 x 128-part chunks), S=512
  free] kept in f32r (fp32 w/ 12-bit mantissa, full PE rate) for the
  residual stream; q/k/v/ctx/probs/inter in bf16 (halves SBUF + 2x DVE/ACT
  eviction rate, full PE rate).
- All weights are bf16 (host-cast), DMAed in large per-matrix tiles
  ([128, 4608] etc). bf16 stationary operands allow walrus to emit separate
  LDWEIGHTS + fast-weight-load, overlapping weight loads with matmul
  streaming (f32r stationaries must self-load serially).
- Matmul dtype purity (walrus rejects f32r x bf16): weight/attention
  matmuls are bf16 x bf16 (weights, xb, q/k/v, ctx, probs, inter); LN
  stat/broadcast matmuls are f32r x f32r. The f32r residual stream never
  passes through a bf16 matmul: PSUM eviction + per-feature bias +
  residual add land in ONE DVE scalar_tensor_tensor op.
- LayerNorm (feature = partition dir) via (1/H)-column matmuls yielding
  mean / mean-of-squares directly (zero LN bias exploited, spec
  fill=zeros); normalize = TT subtract(mean bcast) + STT mult(g) x
  rstd-bcast per chunk; bf16 shadow copies (xb, lo_b) feed the next
  matmuls.
- Attention: scores^T per head via K=64 matmuls, even/odd head pairs
  interleaved so disjoint PE row-groups run concurrently; fused exp
  (scale 1/8, zero mask) -> bf16 probs; ctx via V-token-major stationary
  with appended ones column so the softmax denominator lands in the same
  PSUM tile; per-q normalization via DVE reciprocal + K=1 broadcast matmul.
- FFN interleaved per 128-wide inter chunk: 6 FFN1 matmuls -> fused
  bias+gelu -> bf16 inter -> 6 FFN2 accumulating matmuls (6 PSUM banks).
"""
import os
import sys
import numpy as np

sys.path.insert(0, '/opt/trn_rl_repo')

L, B, S, H, NH, DH, FF = 12, 8, 512, 768, 12, 64, 3072
HC = H // 128      # 6 hidden chunks
FC = FF // 128     # 24 ff chunks
SC = S // 128      # 4 token chunks
EPS = 1e-12
NL = int(os.environ.get("KERNEL_NL", L))   # layers (env override for sim)

_CACHE = {}


def _round_f32r(x):
    b = np.ascontiguousarray(x, np.float32).view(np.uint32)
    r = ((b.astype(np.uint64) + 0x800) & 0xFFFFF000).astype(np.uint32)
    return r.view(np.float32)


def _build_program():
    import concourse.tile as tile
    from concourse import bacc, mybir

    F32 = mybir.dt.float32
    F32R = mybir.dt.float32r
    BF16 = mybir.dt.bfloat16
    AFT = mybir.ActivationFunctionType
    ALU = mybir.AluOpType

    nc = bacc.Bacc("TRN2", target_bir_lowering=False, debug=False)

    # hsT holds f32r-prerounded fp32 bits; declared f32r so DMA is a bit-copy
    hsT = nc.dram_tensor("hsT", [H, S], F32R, kind="ExternalInput").ap()
    Wq = nc.dram_tensor("Wq", [NL, HC, 128, H], BF16, kind="ExternalInput").ap()
    Wk = nc.dram_tensor("Wk", [NL, HC, 128, H], BF16, kind="ExternalInput").ap()
    Wv = nc.dram_tensor("Wv", [NL, HC, 128, H], BF16, kind="ExternalInput").ap()
    Wo = nc.dram_tensor("Wo", [NL, HC, 128, H], BF16, kind="ExternalInput").ap()
    Wi = nc.dram_tensor("Wi", [NL, 4, HC, 128, H], BF16,
                        kind="ExternalInput").ap()
    Wo2 = nc.dram_tensor("Wo2", [NL, HC, 4, 128, H], BF16,
                         kind="ExternalInput").ap()
    # packed per-layer 768-vecs: bq, bk, bo_eff, g1, g2, bo2 -> [NL,128,6*HC]
    vecs = nc.dram_tensor("vecs", [NL, 128, 6 * HC], F32,
                          kind="ExternalInput").ap()
    biv = nc.dram_tensor("biv", [NL, 128, FC], F32, kind="ExternalInput").ap()
    outT = nc.dram_tensor("outT", [H, S], F32R, kind="ExternalOutput").ap()

    trace_sim = bool(os.environ.get("KERNEL_TRACE_SIM"))
    with tile.TileContext(nc, trace_sim=trace_sim) as tc, \
            nc.allow_low_precision(reason="bf16/f32r matmul pipeline"):
        with (
            tc.tile_pool(name="persist", bufs=1) as pp,
            tc.tile_pool(name="wq4", bufs=3) as pw,      # [128,4608] bf16
            tc.tile_pool(name="wi4", bufs=3) as pwi,
            tc.tile_pool(name="wo24", bufs=3) as pwo2,
            tc.tile_pool(name="resid", bufs=2) as prs,   # f32r [128,3072]
            tc.tile_pool(name="probs", bufs=3) as ppr,
            tc.tile_pool(name="inter", bufs=2) as pit,
            tc.tile_pool(name="sq", bufs=3) as psq,
            tc.tile_pool(name="small", bufs=2) as psm,
            tc.tile_pool(name="bias", bufs=2) as pb,
            tc.tile_pool(name="stat", bufs=3) as pst,
            tc.tile_pool(name="psum", bufs=2, space="PSUM") as ps,
        ):
            ones32 = pp.tile([128, 128], F32, tag="ones32", name="ones32")
            nc.vector.memset(ones32[:], 1.0)
            zeros32 = pp.tile([128, 64], F32, tag="zeros32", name="zeros32")
            nc.vector.memset(zeros32[:], 0.0)
            ones = pp.tile([128, 128], F32R, tag="ones", name="ones")
            nc.vector.tensor_copy(ones[:], ones32[:])
            # onec holds 1/H so the LN sum matmuls directly yield mean and
            # mean-of-squares
            invh32 = pp.tile([128, 1], F32, tag="invh32", name="invh32")
            nc.vector.memset(invh32[:], 1.0 / H)
            onec = pp.tile([128, 1], F32R, tag="onec", name="onec")
            nc.vector.tensor_copy(onec[:], invh32[:])
            eps_t = pp.tile([1, 1], F32, tag="eps", name="eps_t")
            nc.vector.memset(eps_t[:], EPS)

            xT = pp.tile([128, HC * 512], F32R, tag="xT", name="xT")
            nc.sync.dma_start(xT[:].rearrange("p (c s) -> p c s", c=HC),
                              hsT.rearrange("(c p) s -> p c s", p=128))
            xb = pp.tile([128, HC * 512], BF16, tag="xb", name="xb")
            for c in range(HC):
                nc.vector.tensor_copy(xb[:, c * 512:(c + 1) * 512],
                                      xT[:, c * 512:(c + 1) * 512])

            qT = pp.tile([128, HC * 512], BF16, tag="qT", name="qT")
            kT = pp.tile([128, HC * 512], BF16, tag="kT", name="kT")
            ctxT = pp.tile([128, HC * 512], BF16, tag="ctxT", name="ctxT")
            loT = pp.tile([128, HC * 512], F32R, tag="loT", name="loT")
            lo_b = pp.tile([128, HC * 512], BF16, tag="lob", name="lo_b")
            # v_tok2: [s-chunk][head][128 cols]; even head [v(64)|1|z63],
            # odd head [1|z63|v(64)]
            vt = pp.tile([128, SC * NH * 128], BF16, tag="vt", name="vt")
            vt4 = vt[:].rearrange("p (sc h c) -> p sc h c", sc=SC, h=NH)
            nc.vector.tensor_copy(
                vt4[:, :, 0::2, 64:65],
                ones32[:, None, None, 0:1].broadcast_to([128, SC, 6, 1]))
            nc.vector.tensor_copy(
                vt4[:, :, 0::2, 65:128],
                zeros32[:, None, None, 0:63].broadcast_to([128, SC, 6, 63]))
            nc.vector.tensor_copy(
                vt4[:, :, 1::2, 0:1],
                ones32[:, None, None, 0:1].broadcast_to([128, SC, 6, 1]))
            nc.vector.tensor_copy(
                vt4[:, :, 1::2, 1:64],
                zeros32[:, None, None, 0:63].broadcast_to([128, SC, 6, 63]))

            def sl(t, c):
                return t[:, c * 512:(c + 1) * 512]

            def stats_chain(nc, S1, S2, tag):
                """S1=mean, S2=mean-of-sq [1,512] PSUM (onec carries 1/H)
                -> mean_r, rstd_r (f32r rows)."""
                mean_r = pst.tile([1, 512], F32R, tag="statr",
                                  name=f"meanr_{tag}")
                nc.vector.tensor_copy(mean_r[:], S1[:])
                msq = pst.tile([1, 512], F32, tag="stat32",
                               name=f"msq_{tag}")
                nc.scalar.activation(msq[:], S1[:], AFT.Square)
                var = pst.tile([1, 512], F32, tag="stat32", name=f"var_{tag}")
                nc.vector.tensor_tensor(var[:], S2[:], msq[:], ALU.subtract)
                sd = pst.tile([1, 512], F32, tag="stat32", name=f"sd_{tag}")
                nc.scalar.activation(sd[:], var[:], AFT.Sqrt,
                                     bias=eps_t[0:1, :])
                rstd_r = pst.tile([1, 512], F32R, tag="statr",
                                  name=f"rstdr_{tag}")
                nc.vector.reciprocal(rstd_r[:], sd[:])
                return mean_r, rstd_r

            for li in range(NL):
                vec_t = pb.tile([128, 6 * HC], F32, tag="vec",
                                name=f"vec_{li}")
                nc.sync.dma_start(vec_t[:], vecs[li])
                bi_t = pb.tile([128, FC], F32, tag="biv", name=f"biv_{li}")
                nc.sync.dma_start(bi_t[:], biv[li])
                def vslot(j, c):
                    return vec_t[:, j * HC + c: j * HC + c + 1]

                wq_t = pw.tile([128, HC * H], BF16, tag="wbig",
                               name=f"wq_{li}")
                nc.sync.dma_start(
                    wq_t[:].rearrange("p (c o) -> p c o", c=HC),
                    Wq[li].rearrange("c p o -> p c o"))
                wk_t = pw.tile([128, HC * H], BF16, tag="wbig",
                               name=f"wk_{li}")
                nc.sync.dma_start(
                    wk_t[:].rearrange("p (c o) -> p c o", c=HC),
                    Wk[li].rearrange("c p o -> p c o"))
                wv_t = pw.tile([128, HC * H], BF16, tag="wbig",
                               name=f"wv_{li}")
                nc.sync.dma_start(
                    wv_t[:].rearrange("p (c o) -> p c o", c=HC),
                    Wv[li].rearrange("c p o -> p c o"))
                wo_t = pw.tile([128, HC * H], BF16, tag="wbig",
                               name=f"wo_{li}")
                nc.sync.dma_start(
                    wo_t[:].rearrange("p (c o) -> p c o", c=HC),
                    Wo[li].rearrange("c p o -> p c o"))

                # ---- Q/K projections, feature-major ----
                for m in range(HC):
                    q_ps = ps.tile([128, 512], F32, tag="ps",
                                   name=f"qps_{li}_{m}")
                    for c in range(HC):
                        nc.tensor.matmul(
                            q_ps[:], wq_t[:, c * H + m * 128:
                                          c * H + (m + 1) * 128],
                            sl(xb, c), start=(c == 0), stop=(c == HC - 1))
                    nc.scalar.activation(sl(qT, m), q_ps[:], AFT.Identity,
                                         bias=vslot(0, m))
                for m in range(HC):
                    k_ps = ps.tile([128, 512], F32, tag="ps",
                                   name=f"kps_{li}_{m}")
                    for c in range(HC):
                        nc.tensor.matmul(
                            k_ps[:], wk_t[:, c * H + m * 128:
                                          c * H + (m + 1) * 128],
                            sl(xb, c), start=(c == 0), stop=(c == HC - 1))
                    nc.scalar.activation(sl(kT, m), k_ps[:], AFT.Identity,
                                         bias=vslot(1, m))

                # ---- V projection, token-major into vt (bv folded into
                # bo_eff); stationary = bf16 x token-slices ----
                for sc in range(SC):
                    for half in range(2):
                        v_ps = ps.tile([128, 384], F32, tag="ps",
                                       name=f"vps_{li}_{sc}_{half}")
                        for c in range(HC):
                            nc.tensor.matmul(
                                v_ps[:],
                                xb[:, c * 512 + sc * 128:
                                   c * 512 + (sc + 1) * 128],
                                wv_t[:, c * H + half * 384:
                                     c * H + (half + 1) * 384],
                                start=(c == 0), stop=(c == HC - 1))
                        v3 = v_ps[:].rearrange("p (h x c) -> p h x c",
                                               h=3, x=2)
                        nc.vector.tensor_copy(
                            vt4[:, sc, half * 6 + 0:half * 6 + 6:2, 0:64],
                            v3[:, :, 0, :])
                        nc.vector.tensor_copy(
                            vt4[:, sc, half * 6 + 1:half * 6 + 6:2, 64:128],
                            v3[:, :, 1, :])

                # ---- attention, head pairs; fused exp; no mask (zero) ----
                att_cm = tc.tile_pool(name=f"att{li}", bufs=1, space="PSUM")
                pat = att_cm.__enter__()
                for hp in range(NH // 2):
                    c = hp
                    pr_eo = [ppr.tile([128, SC * 512], BF16, tag="probs",
                                      name=f"probs_{li}_{2 * hp + r}")
                             for r in range(2)]
                    for half in range(2):
                        st = [pat.tile([128, 1024], F32, tag="satt", bufs=2,
                                       name=f"sps_{li}_{hp}_{r}_{half}")
                              for r in range(2)]
                        # interleave even/odd head matmuls: disjoint PE
                        # row-groups (base partition 0 vs 64) run conc.
                        for kci in range(2):
                            kc = half * 2 + kci
                            for r in range(2):
                                o = r * 64
                                nc.tensor.matmul(
                                    st[r][:, kci * 512:(kci + 1) * 512],
                                    kT[o:o + 64, c * 512 + kc * 128:
                                       c * 512 + (kc + 1) * 128],
                                    qT[o:o + 64, c * 512:(c + 1) * 512],
                                    start=True, stop=True)
                        for r in range(2):
                            nc.scalar.activation(
                                pr_eo[r][:, half * 1024:(half + 1) * 1024],
                                st[r][:], AFT.Exp,
                                scale=float(1.0 / np.sqrt(DH)))
                    ctx_eo = []
                    for r in range(2):
                        h = 2 * hp + r
                        ctx_ps = pat.tile([128, 512], F32, tag="ctx", bufs=2,
                                          name=f"cps_{li}_{h}")
                        for kc in range(SC):
                            lhs = (vt4[:, kc, h, 0:65] if r == 0
                                   else vt4[:, kc, h, 0:128])
                            nc.tensor.matmul(
                                ctx_ps[0:(65 if r == 0 else 128), :], lhs,
                                pr_eo[r][:, kc * 512:(kc + 1) * 512],
                                start=(kc == 0), stop=(kc == SC - 1))
                        ctx_eo.append(ctx_ps)
                    b_eo = pat.tile([128, 1024], F32, tag="satt", bufs=2,
                                    name=f"beo_{li}_{hp}")
                    for r in range(2):
                        h = 2 * hp + r
                        o = r * 64
                        drow = 64 if r == 0 else 0
                        ctx_ps = ctx_eo[r]
                        rec = psm.tile([128, 512], F32R, tag="rec",
                                       name=f"rec_{li}_{h}")
                        nc.vector.reciprocal(rec[drow:drow + 1, :],
                                             ctx_ps[drow:drow + 1, :])
                        b_ps = b_eo[:, r * 512:(r + 1) * 512]
                        nc.tensor.matmul(b_ps, ones[drow:drow + 1, :],
                                         rec[drow:drow + 1, :],
                                         start=True, stop=True)
                        bsb = psm.tile([128, 512], F32, tag="bsb",
                                       name=f"bsb_{li}_{h}")
                        nc.vector.tensor_copy(bsb[:], b_ps)
                        nc.vector.tensor_tensor(
                            ctxT[o:o + 64, c * 512:(c + 1) * 512],
                            ctx_ps[o:o + 64, :], bsb[o:o + 64, :], ALU.mult)
                att_cm.__exit__(None, None, None)

                # ---- attn out proj + residual (PE) + LN1 ----
                ln1_cm = tc.tile_pool(name=f"ln1_{li}", bufs=1, space="PSUM")
                pln = ln1_cm.__enter__()
                axT = prs.tile([128, HC * 512], F32R, tag="resid",
                               name=f"ax_{li}")
                S1 = pln.tile([1, 512], F32, tag="lnp", bufs=4,
                              name=f"s1_{li}")
                S2 = pln.tile([1, 512], F32, tag="lnp", bufs=4,
                              name=f"s2_{li}")
                for m in range(HC):
                    a_ps = ps.tile([128, 512], F32, tag="ps",
                                   name=f"aps_{li}_{m}")
                    for c in range(HC):
                        nc.tensor.matmul(
                            a_ps[:], wo_t[:, c * H + m * 128:
                                          c * H + (m + 1) * 128],
                            sl(ctxT, c), start=(c == 0), stop=(c == HC - 1))
                    # evict + per-feature bias + residual in one DVE op
                    nc.vector.scalar_tensor_tensor(
                        sl(axT, m), a_ps[:], vslot(2, m), sl(xT, m),
                        ALU.add, ALU.add)
                    sq = psq.tile([128, 512], F32R, tag="sq",
                                  name=f"sq1_{li}_{m}")
                    nc.scalar.activation(sq[:], sl(axT, m), AFT.Square)
                    nc.tensor.matmul(S1[:], onec[:, :], sl(axT, m),
                                     start=(m == 0), stop=(m == HC - 1),
                                     skip_group_check=True)
                    nc.tensor.matmul(S2[:], onec[:, :], sq[:],
                                     start=(m == 0), stop=(m == HC - 1),
                                     skip_group_check=True)

                mean_r, rstd_r = stats_chain(nc, S1, S2, f"l1_{li}")
                rb1 = pln.tile([128, 512], F32, tag="lnp", bufs=4,
                               name=f"rb1_{li}")
                nc.tensor.matmul(rb1[:], ones[0:1, :], rstd_r[:],
                                 start=True, stop=True)
                mb1 = pln.tile([128, 512], F32, tag="lnp", bufs=4,
                               name=f"mb1_{li}")
                nc.tensor.matmul(mb1[:], ones[0:1, :], mean_r[:],
                                 start=True, stop=True)
                for m in range(HC):
                    nc.vector.tensor_tensor(sl(axT, m), sl(axT, m), mb1[:],
                                            ALU.subtract)
                    nc.vector.scalar_tensor_tensor(
                        sl(loT, m), sl(axT, m), vslot(3, m), rb1[:],
                        ALU.mult, ALU.mult)
                    nc.scalar.activation(sl(lo_b, m), sl(loT, m), AFT.Copy)
                ln1_cm.__exit__(None, None, None)

                # ---- FFN interleaved; 6 acc banks ----
                ffn_cm = tc.tile_pool(name=f"ffn{li}", bufs=6, space="PSUM")
                pacc = ffn_cm.__enter__()
                acc_ps = [pacc.tile([128, 512], F32, tag="ffacc",
                                    name=f"facc_{li}_{m}")
                          for m in range(HC)]
                for g in range(4):
                    wi_t = pwi.tile([128, HC * H], BF16, tag="wi",
                                    name=f"wi_{li}_{g}")
                    nc.sync.dma_start(
                        wi_t[:].rearrange("p (c o) -> p c o", c=HC),
                        Wi[li, g].rearrange("c p o -> p c o"))
                    for fg in range(HC):
                        f = g * HC + fg
                        if f % 4 == 0:
                            wo2_t = pwo2.tile([128, 4 * H], BF16, tag="wo2",
                                              name=f"wo2_{li}_{f // 4}")
                            nc.sync.dma_start(
                                wo2_t[:].rearrange("p (j o) -> p j o", j=4),
                                Wo2[li, f // 4].rearrange("j p o -> p j o"))
                        f1_ps = ps.tile([128, 512], F32, tag="ps",
                                        name=f"f1_{li}_{f}")
                        for c in range(HC):
                            nc.tensor.matmul(
                                f1_ps[:], wi_t[:, c * H + fg * 128:
                                               c * H + (fg + 1) * 128],
                                sl(lo_b, c), start=(c == 0),
                                stop=(c == HC - 1))
                        inter = pit.tile([128, 512], BF16, tag="inter",
                                         name=f"it_{li}_{f}")
                        nc.scalar.activation(inter[:], f1_ps[:], AFT.Gelu,
                                             bias=bi_t[:, f:f + 1])
                        j = f % 4
                        for m in range(HC):
                            nc.tensor.matmul(
                                acc_ps[m][:],
                                wo2_t[:, j * H + m * 128:
                                      j * H + (m + 1) * 128],
                                inter[:], start=(f == 0),
                                stop=(f == FC - 1),
                                skip_group_check=True)

                # ---- FFN epilogue: +residual, evict, LN2 stats ----
                fxT = prs.tile([128, HC * 512], F32R, tag="resid",
                               name=f"fx_{li}")
                S1b = ps.tile([1, 512], F32, tag="ps", name=f"s1b_{li}")
                S2b = ps.tile([1, 512], F32, tag="ps", name=f"s2b_{li}")
                for m in range(HC):
                    nc.vector.scalar_tensor_tensor(
                        sl(fxT, m), acc_ps[m][:], vslot(5, m), sl(loT, m),
                        ALU.add, ALU.add)
                    sq = psq.tile([128, 512], F32R, tag="sq",
                                  name=f"sq2_{li}_{m}")
                    nc.scalar.activation(sq[:], sl(fxT, m), AFT.Square)
                    nc.tensor.matmul(S1b[:], onec[:, :], sl(fxT, m),
                                     start=(m == 0), stop=(m == HC - 1),
                                     skip_group_check=True)
                    nc.tensor.matmul(S2b[:], onec[:, :], sq[:],
                                     start=(m == 0), stop=(m == HC - 1),
                                     skip_group_check=True)
                ffn_cm.__exit__(None, None, None)

                ln2_cm = tc.tile_pool(name=f"ln2_{li}", bufs=1, space="PSUM")
                pl2 = ln2_cm.__enter__()
                mean2, rstd2 = stats_chain(nc, S1b, S2b, f"l2_{li}")
                rb2 = pl2.tile([128, 512], F32, tag="l2p", bufs=2,
                               name=f"rb2_{li}")
                nc.tensor.matmul(rb2[:], ones[0:1, :], rstd2[:],
                                 start=True, stop=True)
                mb2 = pl2.tile([128, 512], F32, tag="l2p", bufs=2,
                               name=f"mb2_{li}")
                nc.tensor.matmul(mb2[:], ones[0:1, :], mean2[:],
                                 start=True, stop=True)
                for m in range(HC):
                    nc.vector.tensor_tensor(sl(fxT, m), sl(fxT, m), mb2[:],
                                            ALU.subtract)
                    nc.vector.scalar_tensor_tensor(
                        sl(xT, m), sl(fxT, m), vslot(4, m), rb2[:],
                        ALU.mult, ALU.mult)
                    if li < NL - 1:
                        nc.vector.tensor_copy(sl(xb, m), sl(xT, m))
                ln2_cm.__exit__(None, None, None)

            nc.sync.dma_start(outT.rearrange("(c p) s -> p c s", p=128),
                              xT[:].rearrange("p (c s) -> p c s", c=HC))

    nc.compile()
    return nc


def _get_runner():
    if "runner" in _CACHE:
        return _CACHE["runner"]
    import jax
    from jax.sharding import Mesh, PartitionSpec
    from jax.experimental.shard_map import shard_map
    from concourse import mybir
    from concourse.bass2jax import (_bass_exec_p, install_neuronx_cc_hook,
                                    partition_id_tensor)

    install_neuronx_cc_hook()
    nc = _build_program()

    pname = nc.partition_id_tensor.name if nc.partition_id_tensor else None
    in_names, out_names, out_avals, zero_outs = [], [], [], []
    for alloc in nc.m.functions[0].allocations:
        if not isinstance(alloc, mybir.MemoryLocationSet):
            continue
        name = alloc.memorylocations[0].name
        if alloc.kind == "ExternalInput":
            if name == pname:
                continue
            in_names.append(name)
        elif alloc.kind == "ExternalOutput":
            out_names.append(name)
            shape = tuple(alloc.tensor_shape)
            dtype = mybir.dt.np(alloc.dtype)
            out_avals.append(jax.core.ShapedArray(shape, dtype))
            zero_outs.append(np.zeros(shape, dtype))
    n_params = len(in_names)
    n_outs = len(out_avals)
    all_in_names = list(in_names) + list(out_names)
    if pname is not None:
        all_in_names = all_in_names + [pname]

    def _body(*args):
        operands = list(args)
        if pname is not None:
            operands.append(partition_id_tensor())
        outs = _bass_exec_p.bind(
            *operands,
            out_avals=tuple(out_avals),
            in_names=tuple(all_in_names),
            out_names=tuple(out_names),
            lowering_input_output_aliases=(),
            sim_require_finite=False,
            sim_require_nnan=False,
            nc=nc,
        )
        return tuple(outs)

    devices = jax.devices()[:B]
    mesh = Mesh(np.asarray(devices), ("core",))
    in_specs = (PartitionSpec("core"),) * (n_params + n_outs)
    out_specs = (PartitionSpec("core"),) * n_outs
    donate = tuple(range(n_params, n_params + n_outs))
    jitted = jax.jit(
        shard_map(_body, mesh=mesh, in_specs=in_specs, out_specs=out_specs,
                  check_rep=False),
        donate_argnums=donate, keep_unused=True)

    runner = {
        "jit": jitted, "in_names": in_names, "out_names": out_names,
        "zero_outs": zero_outs, "mesh": mesh, "devices": devices,
    }
    _CACHE["runner"] = runner
    return runner


def _prep_core_inputs(inputs):
    import ml_dtypes
    BF = ml_dtypes.bfloat16
    hs = np.asarray(inputs["hidden_states"], np.float32)
    mask = np.asarray(inputs["attention_mask"], np.float32)
    if np.any(mask):
        raise NotImplementedError(
            "kernel compiled for the zero attention_mask this problem "
            "guarantees (spec fill=zeros); nonzero mask unsupported")
    Wq = np.asarray(inputs["Wq"], np.float32)[:NL]
    Wk = np.asarray(inputs["Wk"], np.float32)[:NL]
    Wv = np.asarray(inputs["Wv"], np.float32)[:NL]
    Wo = np.asarray(inputs["Wo"], np.float32)[:NL]
    Wi = np.asarray(inputs["Wi"], np.float32)[:NL]
    Wo2 = np.asarray(inputs["Wo2"], np.float32)[:NL]
    bq = np.asarray(inputs["bq"], np.float32)[:NL]
    bk = np.asarray(inputs["bk"], np.float32)[:NL]
    bv = np.asarray(inputs["bv"], np.float32)[:NL]
    bo = np.asarray(inputs["bo"], np.float32)[:NL]
    bi = np.asarray(inputs["bi"], np.float32)[:NL]
    bo2 = np.asarray(inputs["bo2"], np.float32)[:NL]
    g1 = np.asarray(inputs["ln1_g"], np.float32)[:NL]
    b1 = np.asarray(inputs["ln1_b"], np.float32)[:NL]
    g2 = np.asarray(inputs["ln2_g"], np.float32)[:NL]
    b2 = np.asarray(inputs["ln2_b"], np.float32)[:NL]

    Wq_b = Wq.astype(BF).reshape(NL, HC, 128, H)
    Wk_b = Wk.astype(BF).reshape(NL, HC, 128, H)
    Wv_b = Wv.astype(BF).reshape(NL, HC, 128, H)
    Wo_b = Wo.astype(BF).reshape(NL, HC, 128, H)
    Wi_b = np.ascontiguousarray(
        Wi.astype(BF).reshape(NL, HC, 128, 4, H).transpose(0, 3, 1, 2, 4))
    Wo2_b = Wo2.astype(BF).reshape(NL, HC, 4, 128, H)

    # fold bv into bo: (ctx + bv) @ Wo + bo == ctx @ Wo + (bo + bv @ Wo)
    bo_eff = (bo.astype(np.float64)
              + np.einsum("lh,lho->lo", bv.astype(np.float64),
                          Wo_b.reshape(NL, H, H).astype(np.float64))
              ).astype(np.float32)

    def pack768(v):  # [NL,768] -> [NL,128,HC] with [l,p,c] = v[l, c*128+p]
        return np.ascontiguousarray(v.reshape(NL, HC, 128).transpose(0, 2, 1))

    if np.any(b1) or np.any(b2):
        raise NotImplementedError(
            "kernel compiled for zero LayerNorm biases (spec fill=zeros); "
            "nonzero ln1_b/ln2_b unsupported")
    vecs = np.stack([pack768(v) for v in
                     (bq, bk, bo_eff, g1, g2, bo2)], axis=2)
    vecs = np.ascontiguousarray(vecs.reshape(NL, 128, 6 * HC))
    biv = np.ascontiguousarray(bi.reshape(NL, FC, 128).transpose(0, 2, 1))

    per_core = {
        "hsT": [np.ascontiguousarray(_round_f32r(hs[b].T)) for b in range(B)],
    }
    for name, arr in (("Wq", Wq_b), ("Wk", Wk_b), ("Wv", Wv_b), ("Wo", Wo_b),
                      ("Wi", Wi_b), ("Wo2", Wo2_b), ("vecs", vecs),
                      ("biv", biv)):
        per_core[name] = [arr] * B
    return per_core


def run_on_device(inputs, n_timing_runs=0):
    """Execute; returns (output [B,S,H] fp32, exec_seconds or None)."""
    import jax
    from jax.sharding import NamedSharding, PartitionSpec
    runner = _get_runner()
    per_core = _prep_core_inputs(inputs)
    devices = runner["devices"]
    mesh = runner["mesh"]
    sharding = NamedSharding(mesh, PartitionSpec("core"))

    global_args = []
    for name in runner["in_names"]:
        shards = per_core[name]
        arrs = [jax.device_put(shards[c], devices[c]) for c in range(B)]
        gshape = (B * shards[0].shape[0],) + shards[0].shape[1:]
        global_args.append(
            jax.make_array_from_single_device_arrays(gshape, sharding, arrs))

    def zeros_args():
        outs = []
        for z in runner["zero_outs"]:
            arrs = [jax.device_put(z, devices[c]) for c in range(B)]
            gshape = (B * z.shape[0],) + z.shape[1:]
            outs.append(jax.make_array_from_single_device_arrays(
                gshape, sharding, arrs))
        return outs

    out_arrs = runner["jit"](*global_args, *zeros_args())
    jax.block_until_ready(out_arrs)

    exec_s = None
    if n_timing_runs > 0:
        import time
        times = []
        for _ in range(n_timing_runs):
            zo = zeros_args()
            jax.block_until_ready(zo)
            t0 = time.perf_counter()
            out_arrs = runner["jit"](*global_args, *zo)
            jax.block_until_ready(out_arrs)
            times.append(time.perf_counter() - t0)
        exec_s = min(times)

    outT = np.asarray(out_arrs[0]).reshape(B, H, S)
    out = np.ascontiguousarray(outT.transpose(0, 2, 1))
    return out, exec_s


def kernel(**inputs) -> np.ndarray:
    out, _ = run_on_device(inputs, n_timing_runs=0)
    return out
